# revision 1
# baseline (speedup 1.0000x reference)
"""Causal linear attention (chunked scan) for Trainium2, 8 NeuronCores.

Problem: B=4, T=2048, E=1024, H=16 heads, D=64, CHUNK=128.
  qkv = x @ w_attn.T ; q,k -> phi = elu+1 ; chunked causal linear attention
  with running state S[D,D], z[D] per (b,h); out = y @ w_proj.T.

Sharding: core = b*2 + hg  (b in 0..3 batches, hg in 0..1 half-of-heads).
Each core computes its batch's tokens against its 8 heads; the host sums the
two half-head partial outputs per batch. No cross-core traffic.

Single interleaved pipeline per core (emission order = PE order):
  proj(tb0), then for tb: proj(tb) followed by scan chunks of tb-1, with the
  output-projection block of chunk n-1 emitted inside chunk n as PE filler
  while the DVE mask/cast of chunk n runs.

PSUM (8 banks): proj/p3/sdb share a 2-buf [128,512]f32 tag; transposes share
a 1-buf [128,128]bf16 tag; scores 2 banks; num 2 banks. Per-head score/num
regions are parity-segregated by bank: the HW rejects two different partial
row-group matmuls (K=64@0 / K=64@64) writing the same PSUM bank.
"""

import sys

if "/opt/trn_rl_repo" not in sys.path:
    sys.path.insert(0, "/opt/trn_rl_repo")

import numpy as np
import ml_dtypes

B, T, E = 4, 2048, 1024
H, D, CH = 16, 64, 128
NCH = T // CH            # 16 chunks
HPC = H // 2             # 8 heads per core
EB = E // 128            # 8 contraction blocks
FB_QK = 8                # feature blocks for q|k (1024 features)
TB = 4                   # token blocks of 512 for the A-part
N_CORES = 8

_cache = {}


def _build():
    import concourse.bacc as bacc
    import concourse.tile as tile
    from concourse import mybir

    f32 = mybir.dt.float32
    bf16 = mybir.dt.bfloat16
    AF = mybir.ActivationFunctionType

    nc = bacc.Bacc("TRN2", target_bir_lowering=False, debug=False, num_devices=N_CORES)

    xT = nc.dram_tensor("xT", [E, T], bf16, kind="ExternalInput")
    wA = nc.dram_tensor("wA", [E, 1024], bf16, kind="ExternalInput")
    wB = nc.dram_tensor("wB", [E, 512], bf16, kind="ExternalInput")
    wpT = nc.dram_tensor("wpT", [512, E], bf16, kind="ExternalInput")
    maskT = nc.dram_tensor("maskT", [CH, HPC * CH], bf16, kind="ExternalInput")
    ident = nc.dram_tensor("ident", [CH, CH], bf16, kind="ExternalInput")
    out = nc.dram_tensor("out", [T, E], f32, kind="ExternalOutput")

    with tile.TileContext(nc) as tc:
        with tc.tile_pool(name="main", bufs=1) as main, \
             tc.tile_pool(name="phist", bufs=4) as phist, \
             tc.tile_pool(name="scmp", bufs=4) as scmp, \
             tc.tile_pool(name="st2", bufs=4) as st2, \
             tc.tile_pool(name="ost", bufs=4) as ost, \
             tc.tile_pool(name="psProj", bufs=2, space="PSUM") as psProj, \
             tc.tile_pool(name="psTp", bufs=1, space="PSUM") as psTp, \
             tc.tile_pool(name="psSd", bufs=1, space="PSUM") as psSd, \
             tc.tile_pool(name="psS", bufs=2, space="PSUM") as psS, \
             tc.tile_pool(name="psN", bufs=2, space="PSUM") as psN:
            qkT = main.tile([128, FB_QK * T], bf16)
            k_tok = main.tile([128, NCH * 512], bf16)
            v_aug = main.tile([128, NCH * HPC * 65], bf16)
            mask_sb = main.tile([128, HPC * CH], bf16)
            id_sb = main.tile([128, CH], bf16)
            S_sb = main.tile([128, 4 * 128], bf16)
            S_f32 = main.tile([128, 4 * 128], f32)
            yT_all = main.tile([128, 4 * T], bf16)
            wp_sb = main.tile([128, 4 * 1024], bf16)
            xT_sb = main.tile([128, EB * T], bf16)
            wA_sb = main.tile([128, EB * 1024], bf16)
            wB_sb = main.tile([128, EB * 512], bf16)

            nc.sync.dma_start(mask_sb[:], maskT[:])
            nc.sync.dma_start(id_sb[:], ident[:])
            nc.vector.memset(S_sb[:], 0.0)
            nc.vector.memset(S_f32[:], 0.0)
            ones_view = v_aug.rearrange("p (n h e) -> p n h e", h=HPC, e=65)[:, :, :, 64]
            nc.vector.memset(ones_view, 1.0)

            # stream the tb0 slices of xT + all of wA first so the first
            # A-units (which accumulate over every eb) unblock after ~3 MB,
            # not 6 MB; later token blocks, wB, wpT follow
            for eb in range(EB):
                nc.sync.dma_start(wB_sb[:, eb * 512:(eb + 1) * 512],
                                  wB[eb * 128:(eb + 1) * 128, :])
                nc.sync.dma_start(xT_sb[:, eb * T: eb * T + 512],
                                  xT[eb * 128:(eb + 1) * 128, 0:512])
            for eb in range(EB):
                nc.sync.dma_start(wA_sb[:, eb * 1024: eb * 1024 + 512],
                                  wA[eb * 128:(eb + 1) * 128, 0:512])
            for eb in range(EB):
                nc.sync.dma_start(wA_sb[:, eb * 1024 + 512:(eb + 1) * 1024],
                                  wA[eb * 128:(eb + 1) * 128, 512:1024])
            for tb in range(1, TB):
                for eb in range(EB):
                    nc.sync.dma_start(
                        xT_sb[:, eb * T + tb * 512: eb * T + (tb + 1) * 512],
                        xT[eb * 128:(eb + 1) * 128, tb * 512:(tb + 1) * 512])
            nc.sync.dma_start(wp_sb.rearrange("p (hp t) -> p hp t", hp=4),
                              wpT.rearrange("(hp p) t -> p hp t", p=128))

            def proj_units(tb):
                units = []

                def a_unit(fb, tb=tb):
                    ps = psProj.tile([128, 512], f32, name="psa", tag="proj")
                    for eb in range(EB):
                        nc.tensor.matmul(
                            ps[:],
                            wA_sb[:, eb * 1024 + fb * 128: eb * 1024 + fb * 128 + 128],
                            xT_sb[:, eb * T + tb * 512: eb * T + tb * 512 + 512],
                            start=(eb == 0), stop=(eb == EB - 1))
                    # phi(x) = relu(x) + min(exp(x), 1)
                    ex = phist.tile([128, 512], bf16, name="ex", tag="ex")
                    nc.scalar.activation(ex[:], ps[:], AF.Exp)
                    rl = phist.tile([128, 512], bf16, name="rl", tag="rl")
                    nc.scalar.activation(rl[:], ps[:], AF.Relu)
                    em = phist.tile([128, 512], bf16, name="em", tag="em")
                    nc.vector.tensor_scalar_min(em[:], ex[:], 1.0)
                    dst = qkT[:, fb * T + tb * 512: fb * T + tb * 512 + 512]
                    nc.vector.tensor_add(dst, em[:], rl[:])

                def b_unit(n):
                    ps = psProj.tile([128, 512], f32, name="psb", tag="proj")
                    for eb in range(EB):
                        nc.tensor.matmul(
                            ps[:],
                            xT_sb[:, eb * T + n * CH: eb * T + n * CH + CH],
                            wB_sb[:, eb * 512:(eb + 1) * 512],
                            start=(eb == 0), stop=(eb == EB - 1))
                    src_ = ps.rearrange("p (h e) -> p h e", e=64)
                    dst = v_aug[:, n * HPC * 65:(n + 1) * HPC * 65] \
                        .rearrange("p (h e) -> p h e", e=65)[:, :, 0:64]
                    nc.scalar.copy(dst, src_)

                def kt_unit(n):
                    kt = psTp.tile([128, 512], bf16, name="ktp", tag="tp")
                    for fb4 in range(4):
                        nc.tensor.transpose(
                            kt[:, fb4 * 128:(fb4 + 1) * 128],
                            qkT[:, (4 + fb4) * T + n * CH:(4 + fb4) * T + n * CH + CH],
                            id_sb[:])
                    nc.vector.tensor_copy(
                        k_tok[:, n * 512:(n + 1) * 512], kt[:])

                if tb == 0:
                    for n in range(4):
                        units.append(lambda n=n: b_unit(n))
                    for fb in range(FB_QK):
                        units.append(lambda fb=fb: a_unit(fb))
                else:
                    for fb in range(FB_QK):
                        units.append(lambda fb=fb: a_unit(fb))
                    for n in range(tb * 4, tb * 4 + 4):
                        units.append(lambda n=n: b_unit(n))
                for n in range(tb * 4, tb * 4 + 4):
                    units.append(lambda n=n: kt_unit(n))
                return units

            def emit_p3(n):
                # output projection for token block n (needs yT_all of chunk n)
                ob = ost.tile([128, 1024], f32, name="ob", tag="ob")
                for eo in range(2):
                    po = psProj.tile([128, 512], f32, name="pop", tag="proj")
                    for hp in range(4):
                        nc.tensor.matmul(
                            po[:],
                            yT_all[:, hp * T + n * CH: hp * T + n * CH + CH],
                            wp_sb[:, hp * 1024 + eo * 512: hp * 1024 + eo * 512 + 512],
                            start=(hp == 0), stop=(hp == 3))
                    if eo == 0:
                        nc.scalar.copy(ob[:, 0:512], po[:])
                    else:
                        nc.scalar.copy(ob[:, 512:1024], po[:])
                nc.sync.dma_start(out[n * CH:(n + 1) * CH, :], ob[:])

            def emit_chunk(n, filler=None):
                # scoresT, one [128,512] tile per parity half (bank-pure
                # row-groups; 2-buf rotation pipelines across chunks)
                scm = scmp.tile([128, HPC * CH], bf16, name="scm", tag="scm")
                scgs = [psS.tile([128, 512], f32, name=f"scg{g}", tag="scg")
                        for g in range(2)]
                for hp in range(4):
                    for g in range(2):  # alternate row groups -> PE overlaps
                        b64 = g * 64
                        nc.tensor.matmul(
                            scgs[g][:, hp * CH:(hp + 1) * CH],
                            qkT[b64:b64 + 64,
                                (4 + hp) * T + n * CH:(4 + hp) * T + n * CH + CH],
                            qkT[b64:b64 + 64, hp * T + n * CH: hp * T + n * CH + CH],
                            start=True, stop=True)
                for g in range(2):
                    nc.vector.tensor_mul(scm[:, g * 512:(g + 1) * 512], scgs[g][:],
                                         mask_sb[:, g * 512:(g + 1) * 512])

                # PE filler while the mask/cast runs on DVE
                if n >= 1:
                    emit_p3(n - 1)
                if filler is not None:
                    filler()

                # num = scores @ v_aug + q @ S_aug (per parity half)
                nmb_g = []
                for g in range(2):
                    nmg = psN.tile([128, 512], f32, name="nmg", tag="nmg")
                    nmb_g.append(nmg)
                    for hp in range(4):
                        h = hp * 2 + g
                        b64 = g * 64
                        hc = hp * 128
                        qT_ap = qkT[b64:b64 + 64, hp * T + n * CH: hp * T + n * CH + CH]
                        va_ap = v_aug[:, (n * HPC + h) * 65:(n * HPC + h) * 65 + 65]
                        nc.tensor.matmul(nmg[:, hc: hc + 65],
                                         scm[:, g * 512 + hc: g * 512 + hc + CH], va_ap,
                                         start=True, stop=False)
                        nc.tensor.matmul(nmg[:, hc: hc + 65], qT_ap,
                                         S_sb[b64:b64 + 64, hp * 128: hp * 128 + 65],
                                         start=False, stop=True)

                # state deltas + f32 accumulate + bf16 snapshot
                sdb = psSd.tile([128, 512], f32, name="sdb", tag="sdb")
                for h in range(HPC):
                    b64 = (h % 2) * 64
                    hp = h // 2
                    va_ap = v_aug[:, (n * HPC + h) * 65:(n * HPC + h) * 65 + 65]
                    nc.tensor.matmul(
                        sdb[b64:b64 + 64, hp * 128: hp * 128 + 65],
                        k_tok[:, n * 512 + h * 64: n * 512 + h * 64 + 64],
                        va_ap, start=True, stop=True)
                Sf_v = S_f32.rearrange("p (g e) -> p g e", e=CH)[:, :, 0:65]
                Sb_v = S_sb.rearrange("p (g e) -> p g e", e=CH)[:, :, 0:65]
                sd_v = sdb.rearrange("p (g e) -> p g e", e=CH)[:, :, 0:65]
                nc.vector.tensor_add(Sf_v, Sf_v, sd_v)
                nc.scalar.copy(Sb_v, Sf_v)

                # y = num / den, per parity half
                yb = st2.tile([128, 512], bf16, name="yb", tag="yb")
                yb_v = yb.rearrange("p (hh two e) -> p hh two e", two=2, e=64)
                for g in range(2):
                    half = nmb_g[g].rearrange("p (hh e) -> p hh e", e=CH)
                    rcp = st2.tile([128, 4], f32, name=f"rcp{g}", tag=f"rcp{g}")
                    nc.vector.reciprocal(rcp[:], half[:, :, 64])
                    nc.vector.tensor_mul(
                        yb_v[:, :, g, :],
                        half[:, :, 0:64],
                        rcp[:, :, None].broadcast_to([128, 4, 64]))

                # yT via PE transpose (head pairs) -> yT_all
                ytp = psTp.tile([128, 512], bf16, name="ytp", tag="tp")
                for hp in range(4):
                    nc.tensor.transpose(ytp[:, hp * 128:(hp + 1) * 128],
                                        yb[:, hp * CH:(hp + 1) * CH], id_sb[:])
                nc.scalar.copy(
                    yT_all.rearrange("p (hp t) -> p hp t", hp=4)[:, :, n * CH:(n + 1) * CH],
                    ytp.rearrange("p (hp e) -> p hp e", hp=4))

            for u in proj_units(0):
                u()
            units = []
            for tb in range(1, TB):
                units.extend(proj_units(tb))
            state = {"ui": 0}

            def pace(target):
                while state["ui"] < min(target, len(units)):
                    units[state["ui"]]()
                    state["ui"] += 1

            for n in range(NCH):
                pace(16 * (n // 4))          # hard dep: chunk n needs its tb
                emit_chunk(n, lambda: pace(7 * (n + 1)))
            emit_p3(NCH - 1)

    nc.compile()
    return nc


def _prep_core_inputs(x, w_attn, w_proj, core):
    b, hg = core // 2, core % 2
    s = slice(hg * 512, (hg + 1) * 512)
    xT = np.ascontiguousarray(x[b].T).astype(ml_dtypes.bfloat16)
    wA = np.ascontiguousarray(
        np.concatenate([w_attn[s, :], w_attn[E + hg * 512: E + (hg + 1) * 512, :]], 0).T
    ).astype(ml_dtypes.bfloat16)
    wB = np.ascontiguousarray(
        w_attn[2 * E + hg * 512: 2 * E + (hg + 1) * 512, :].T).astype(ml_dtypes.bfloat16)
    wpT_ = np.ascontiguousarray(w_proj[:, s].T).astype(ml_dtypes.bfloat16)
    mask1 = np.triu(np.ones((CH, CH), dtype=np.float32))
    maskT = np.tile(mask1, (1, HPC)).astype(ml_dtypes.bfloat16)
    ident = np.eye(CH, dtype=np.float32).astype(ml_dtypes.bfloat16)
    return {"xT": xT, "wA": wA, "wB": wB, "wpT": wpT_,
            "maskT": maskT, "ident": ident}


def kernel(x, w_attn, w_proj, _trace=False):
    from concourse.bass_utils import run_bass_kernel_spmd

    if "nc" not in _cache:
        _cache["nc"] = _build()
    nc = _cache["nc"]

    x = np.asarray(x, dtype=np.float32)
    w_attn = np.asarray(w_attn, dtype=np.float32)
    w_proj = np.asarray(w_proj, dtype=np.float32)

    in_maps = [_prep_core_inputs(x, w_attn, w_proj, c) for c in range(N_CORES)]
    res = run_bass_kernel_spmd(nc, in_maps, core_ids=list(range(N_CORES)),
                               trace=_trace)
    _cache["last_results"] = res

    out = np.empty((B, T, E), dtype=np.float32)
    for b in range(B):
        out[b] = res.results[2 * b]["out"] + res.results[2 * b + 1]["out"]
    return out

